# revision 15
# baseline (speedup 1.0000x reference)
"""Trainium2 Bass kernel for LayerNorm + multi-head attention (B=4, S=2048,
D=1024, H=16) with sigmoid(s-mu)*exp(s) row-normalized attention weights.

Sharding: 8 cores = 4 batches x 2 head-groups (8 heads each). Each core
computes LN + its head-group's q/k/v projections + attention + a partial
output projection; the host sums the two partials per batch and adds bo.

Device-side identities:
  p_i = g(z_i) / sum_j g(z_j),  z = s - mu,  g(z) = e^z * sigmoid(z)
  1 / g(z) = y * (1 + y)  with  y = e^{-z}
so per score element: ONE ScalarE exp + ONE fused custom-DVE op computing
1/(x + x^2) (bitwise-not reciprocal seed + one Newton step).

v3 layout/scheduling:
  - all matmul operands bf16; ONE 8-bank PSUM pool for the whole program
    (tags "sc" + "av", 2 bufs each, [128,2,512] f32 = 2 banks per tile)
  - prefix: per token tile LN -> PE transposes -> v projection, fully
    interleaved; only the hp=0 q/k projections gate the start of attention
  - score matmuls for the even/odd head of a pair are K=64 row-tiled into
    the two banks of one "sc" tile; ONE exp + ONE recip-act2 call covers
    both parities (1024 cols)
  - AV matmuls trail the score matmuls by LAG k-tiles so the in-order PE
    queue never stalls on the DVE; q/k projections for hp+1 are injected
    into phase B's PE slack; phase C runs per 512-token block as soon as
    the last head pair's normalization lands
  - normalization: one ACT copy moves values+denominator rows to SBUF,
    one DVE fast-reciprocal, DMA broadcast via a DRAM bounce row, final
    multiplies on the (otherwise idle) Pool/GpSimd engine
"""

import sys

if "/opt/trn_rl_repo" not in sys.path:
    sys.path.insert(0, "/opt/trn_rl_repo")

import numpy as np
import ml_dtypes as _ml

D_MODEL = 1024
N_HEADS = 16
HEAD_DIM = 64
SEQ = 2048
BATCH = 4
N_CORES = 8
EPS = 1e-6
SCALE = float(HEAD_DIM) ** 0.5  # 8.0

# Per-core partitioning
H_LOC = 8          # heads per core
W_LOC = H_LOC * HEAD_DIM  # 512 local projection width
N_DT = D_MODEL // 128     # 8 d-tiles
N_TT = SEQ // 128         # 16 token tiles (t2 direction)
VSTR = 66                 # per-head stride in v_aug ([64 v | 1 ones | 1 pad])
LAG = 2                   # AV matmuls trail score matmuls by LAG k-tiles

# Custom-DVE fused reciprocal-of-act2 constants.
RA_C0 = -0.234
RA_C1 = 2.0


def _get_recip_act2_op():
    """Register (once) and return the custom DVE op: out = 1/(x + x^2)."""
    import concourse.dve_ops as dve_ops

    if hasattr(dve_ops, "RECIP_ACT2_ANT"):
        return dve_ops.RECIP_ACT2_ANT

    from concourse.dve_spec import Spec, Src0, C0, C1, Bin, AluOp, sq, lower, _has_src1
    from concourse.dve_uop import DveOpSpec

    _w = sq(Src0) + Src0
    _nw = Bin(AluOp.BITWISE_NOT, _w, _w)
    _y0 = _nw * C0
    _body = _y0 * (C1 - _w * _y0)

    def _ref(in0, in1, s0, s1, imm2):
        x = np.asarray(in0).astype(np.float32)
        w = (x + x * x).astype(np.float32)
        nw = (~w.view(np.int32)).view(np.float32)
        if isinstance(s0, np.ndarray):
            s0 = s0.astype(np.float32)
        if isinstance(s1, np.ndarray):
            s1 = s1.astype(np.float32)
        y0 = (nw * np.float32(s0) if not isinstance(s0, np.ndarray) else nw * s0).astype(np.float32)
        c1 = np.float32(s1) if not isinstance(s1, np.ndarray) else s1
        return (y0 * (c1 - w * y0)).astype(np.float32)

    spec = Spec(body=_body, reference=_ref)
    name = "RECIP_ACT2_ANT"
    row = max(dve_ops._SUB_OPCODE_FOR_NAME.values()) + 1
    assert row < 0x20
    dve_ops._SUB_OPCODE_FOR_NAME[name] = row
    shas = {}
    for ver in ("v3", "v4"):
        compiled = DveOpSpec(
            name=name, opcode=row, uops=lower(spec, ver=ver), rd1_en=_has_src1(spec)
        )
        shas[ver] = compiled.sha(ver)
    op = dve_ops.DveOp(name, spec, subdim=False, uops_sha=shas)
    dve_ops.OPS.append(op)
    dve_ops.CUSTOM_DVE_SPECS[name] = spec
    dve_ops.RECIP_ACT2_ANT = op
    return op


def _broadcast_ap(ap, parts):
    """Partition-broadcast a 1-D DRAM AP of shape [N] to [parts, N]."""
    import concourse.bass as bass

    steps = [list(p) for p in ap.ap]
    return bass.AP(tensor=ap.tensor, offset=ap.offset, ap=[[0, parts]] + steps)


def _broadcast_row(ap2d, parts):
    """Partition-broadcast a [1, N] DRAM AP to [parts, N]."""
    import concourse.bass as bass

    steps = [list(p) for p in ap2d.ap[1:]]
    return bass.AP(tensor=ap2d.tensor, offset=ap2d.offset, ap=[[0, parts]] + steps)


def _build_program(mu_val: float):
    import concourse.mybir as mybir
    import concourse.tile as tile
    from concourse import bacc
    from concourse.masks import make_identity
    from concourse.dve_ops import RECIPROCAL_APPROX_FAST, RECIP_APPROX_FAST_CONSTS

    recip_act2 = _get_recip_act2_op()

    f32 = mybir.dt.float32
    bf16 = mybir.dt.bfloat16
    AF = mybir.ActivationFunctionType
    ALU = mybir.AluOpType

    nc = bacc.Bacc("TRN2", target_bir_lowering=False, debug=False,
                   num_devices=N_CORES)

    x_d = nc.dram_tensor("x", [SEQ, D_MODEL], bf16, kind="ExternalInput").ap()
    wq_d = nc.dram_tensor("wqT", [D_MODEL, W_LOC], bf16, kind="ExternalInput").ap()
    wk_d = nc.dram_tensor("wkT", [D_MODEL, W_LOC], bf16, kind="ExternalInput").ap()
    wv_d = nc.dram_tensor("wvT", [D_MODEL, W_LOC], bf16, kind="ExternalInput").ap()
    wo_d = nc.dram_tensor("woT", [W_LOC, D_MODEL], bf16, kind="ExternalInput").ap()
    bq_d = nc.dram_tensor("bq", [W_LOC], f32, kind="ExternalInput").ap()
    bk_d = nc.dram_tensor("bk", [W_LOC], f32, kind="ExternalInput").ap()
    bv_d = nc.dram_tensor("bv", [W_LOC], f32, kind="ExternalInput").ap()
    out_d = nc.dram_tensor("out", [SEQ, D_MODEL], f32, kind="ExternalOutput").ap()
    # DRAM bounce rows for the per-block reciprocals (DMA partition-broadcast
    # needs a DRAM source); one row per (hp, t1b) block.
    rsc_d = nc.dram_tensor("rscratch", [16, 2, 512], bf16, kind="Internal").ap()

    rc = RECIP_APPROX_FAST_CONSTS

    with tile.TileContext(nc) as tc:
        with (
            tc.tile_pool(name="consts", bufs=1) as consts,
            tc.tile_pool(name="qkv", bufs=1) as qkv_pool,
            tc.tile_pool(name="xp", bufs=4) as xp,
            tc.tile_pool(name="sp", bufs=6) as sp,
            tc.tile_pool(name="yb", bufs=3) as yp,
            tc.tile_pool(name="gb", bufs=LAG + 2) as gp,
            tc.tile_pool(name="nrm", bufs=2) as nrm,
            tc.tile_pool(name="ob", bufs=3) as op_,
            tc.tile_pool(name="ps8", bufs=2, space="PSUM") as ps8,
        ):
            ident = consts.tile([128, 128], bf16)
            make_identity(nc, ident)
            eps_sb = consts.tile([128, 1], f32)
            nc.vector.memset(eps_sb, EPS)
            bq_sb = consts.tile([128, 4], f32)
            nc.sync.dma_start(out=bq_sb, in_=bq_d.rearrange("(a p) -> p a", p=128))
            bk_sb = consts.tile([128, 4], f32)
            nc.sync.dma_start(out=bk_sb, in_=bk_d.rearrange("(a p) -> p a", p=128))
            bv_bc = consts.tile([128, W_LOC], f32)
            nc.sync.dma_start(out=bv_bc, in_=_broadcast_ap(bv_d, 128))

            # persistent activations (all bf16)
            qT = qkv_pool.tile([128, 4, SEQ], bf16)   # [pair-dim, hp, t]
            kT = qkv_pool.tile([128, 4, SEQ], bf16)
            v_aug = qkv_pool.tile([128, N_TT, H_LOC * VSTR], bf16)
            attn_sb = qkv_pool.tile([128, 4, SEQ], bf16)
            nc.vector.memset(
                v_aug.rearrange("p t (h c) -> p t h c", c=VSTR)[:, :, :, 64:66], 1.0
            )

            wq_sb = qkv_pool.tile([128, N_DT, W_LOC], bf16)
            nc.sync.dma_start(out=wq_sb, in_=wq_d.rearrange("(a p) j -> p a j", p=128))
            wk_sb = qkv_pool.tile([128, N_DT, W_LOC], bf16)
            nc.sync.dma_start(out=wk_sb, in_=wk_d.rearrange("(a p) j -> p a j", p=128))
            wv_sb = qkv_pool.tile([128, N_DT, W_LOC], bf16)
            nc.sync.dma_start(out=wv_sb, in_=wv_d.rearrange("(a p) j -> p a j", p=128))
            wo_sb = qkv_pool.tile([128, 4, D_MODEL], bf16)
            nc.sync.dma_start(out=wo_sb, in_=wo_d.rearrange("(a p) e -> p a e", p=128))
            xT = qkv_pool.tile([128, N_DT, SEQ], bf16)

            def emit_qk_proj(hp, which, cb):
                w_sb, b_sb, dstT = (
                    (wq_sb, bq_sb, qT) if which == 0 else (wk_sb, bk_sb, kT))
                ps_p = ps8.tile([128, 2, 512], f32, tag="sc", name="ps_p")
                for d in range(N_DT):
                    nc.tensor.matmul(
                        ps_p[:, 0, :],
                        w_sb[:, d, hp * 128:(hp + 1) * 128],
                        xT[:, d, cb * 512:(cb + 1) * 512],
                        start=(d == 0), stop=(d == N_DT - 1),
                    )
                nc.scalar.activation(
                    out=dstT[:, hp, cb * 512:(cb + 1) * 512], in_=ps_p[:, 0, :],
                    func=AF.Identity, bias=b_sb[:, hp:hp + 1], scale=1.0)

            qk_pending = [(hp, w, cb) for cb in range(4)
                          for hp in range(4) for w in range(2)]
            qk_done = set()

            try_step_b, finish_b = _phase_b(
                tc, nc, mybir, qT, kT, v_aug, attn_sb, rsc_d, mu_val,
                recip_act2, RECIPROCAL_APPROX_FAST, rc,
                ps8, yp, gp, nrm, op_, wo_sb, out_d, qk_done)

            # ---------------- prefix: LN + transpose + v-proj, interleaved --
            for tt in range(N_TT):
                x_t = xp.tile([128, D_MODEL], bf16, tag="x")
                nc.sync.dma_start(out=x_t, in_=x_d[tt * 128:(tt + 1) * 128, :])
                stats = sp.tile([128, 2, 6], f32, tag="bn")
                nc.vector.bn_stats(out=stats[:, 0, :], in_=x_t[:, 0:512])
                nc.vector.bn_stats(out=stats[:, 1, :], in_=x_t[:, 512:1024])
                mv = sp.tile([128, 2], f32, tag="mv")
                nc.vector.bn_aggr(out=mv, in_=stats)
                rstd = sp.tile([128, 1], f32, tag="rstd")
                nc.scalar.activation(out=rstd, in_=mv[:, 1:2], func=AF.Sqrt,
                                     bias=eps_sb, scale=1.0)
                nc.vector.reciprocal(out=rstd, in_=rstd)
                nmr = sp.tile([128, 1], f32, tag="nmr")
                nc.vector.tensor_scalar(out=nmr, in0=mv[:, 0:1], scalar1=rstd,
                                        scalar2=-1.0, op0=ALU.mult, op1=ALU.mult)
                xs_t = xp.tile([128, D_MODEL], bf16, tag="xs")
                nc.scalar.activation(out=xs_t, in_=x_t, func=AF.Identity,
                                     scale=rstd, bias=nmr)
                ps_tr8 = ps8.tile([128, 8, 128], bf16, tag="sc", name="ps_tr8")
                for d in range(N_DT):
                    nc.tensor.transpose(ps_tr8[:, d, :],
                                        xs_t[:, d * 128:(d + 1) * 128], ident)
                if tt % 2 == 0:
                    nc.scalar.copy(out=xT[:, :, tt * 128:(tt + 1) * 128], in_=ps_tr8)
                else:
                    nc.vector.tensor_copy(out=xT[:, :, tt * 128:(tt + 1) * 128],
                                          in_=ps_tr8)
                # v projection for this token tile
                ps_v = ps8.tile([128, 2, 512], f32, tag="sc", name="ps_v")
                for d in range(N_DT):
                    nc.tensor.matmul(
                        ps_v[:, 0, :],
                        xT[:, d, tt * 128:(tt + 1) * 128],
                        wv_sb[:, d, :],
                        start=(d == 0), stop=(d == N_DT - 1),
                    )
                v_view = v_aug.rearrange("p t (h c) -> p t h c", c=VSTR)[:, tt, :, 0:64]
                nc.vector.tensor_tensor(
                    out=v_view,
                    in0=ps_v[:, 0, :].rearrange("p (h c) -> p h c", c=64),
                    in1=bv_bc.rearrange("p (h c) -> p h c", c=64), op=ALU.add)
                # spread q/k projection tiles over ready chunks
                budget = 3
                while (qk_pending and budget > 0
                       and qk_pending[0][2] * 4 + 3 <= tt):
                    hp_, w_, cb_ = qk_pending.pop(0)
                    emit_qk_proj(hp_, w_, cb_)
                    qk_done.add((hp_, w_, cb_))
                    budget -= 1
                for _ in range(2):
                    if not try_step_b(tt):
                        break

            for hp_, w_, cb_ in qk_pending:
                emit_qk_proj(hp_, w_, cb_)
                qk_done.add((hp_, w_, cb_))
            qk_pending = []
            finish_b()

    nc.compile()
    return nc


def _phase_b(tc, nc, mybir, qT, kT, v_aug, attn_sb, rsc_d, mu_val,
             recip_act2, RECIPROCAL_APPROX_FAST, rc,
             ps8, yp, gp, nrm, op_, wo_sb, out_d, qk_done):
    f32 = mybir.dt.float32
    bf16 = mybir.dt.bfloat16
    AF = mybir.ActivationFunctionType
    ALU = mybir.AluOpType

    blocks = [(hp, t1b) for hp in range(4) for t1b in range(4)]
    n_steps = len(blocks) * N_TT

    g_tiles = {}
    av_tiles = {}

    def emit_lead(step):
        bi, t2t = divmod(step, N_TT)
        hp, t1b = blocks[bi]
        t1s = slice(t1b * 512, (t1b + 1) * 512)
        t2s = slice(t2t * 128, (t2t + 1) * 128)
        ps = ps8.tile([128, 2, 512], f32, tag="sc", name="ps_s")
        nc.tensor.matmul(ps[:, 0, :], kT[0:64, hp, t2s], qT[0:64, hp, t1s],
                         start=True, stop=True)
        nc.tensor.matmul(ps[:, 1, :], kT[64:128, hp, t2s], qT[64:128, hp, t1s],
                         start=True, stop=True)
        y_t = yp.tile([128, 1024], bf16, tag="y")
        nc.scalar.activation(out=y_t, in_=ps.rearrange("p a b -> p (a b)"),
                             func=AF.Exp, scale=-1.0 / SCALE, bias=mu_val)
        g_t = gp.tile([128, 1024], bf16, tag="g")
        nc.vector._custom_dve(recip_act2, out=g_t, in0=y_t, s0=RA_C0, s1=RA_C1)
        g_tiles[step] = g_t

    def emit_trail(step):
        bi, t2t = divmod(step, N_TT)
        hp, t1b = blocks[bi]
        g_t = g_tiles.pop(step)
        if t2t == 0:
            av_tiles[bi] = ps8.tile([128, 2, 512], f32, tag="av", name="av")
        av = av_tiles[bi]
        h_e, h_o = 2 * hp, 2 * hp + 1
        nc.tensor.matmul(
            av[0:65, 0, :], v_aug[:, t2t, h_e * VSTR:h_e * VSTR + 65],
            g_t[:, 0:512],
            start=(t2t == 0), stop=(t2t == N_TT - 1))
        nc.tensor.matmul(
            av[0:65, 1, :], v_aug[:, t2t, h_o * VSTR:h_o * VSTR + 65],
            g_t[:, 512:1024],
            start=(t2t == 0), stop=(t2t == N_TT - 1))
        if t2t == N_TT - 1:
            emit_norm(bi)
            if blocks[bi][0] == 3:
                c_queue.extend(range(4 * blocks[bi][1], 4 * blocks[bi][1] + 4))

    def emit_norm(bi):
        hp, t1b = blocks[bi]
        t1s = slice(t1b * 512, (t1b + 1) * 512)
        av = av_tiles.pop(bi)
        sb_av = nrm.tile([65, 2, 512], f32, tag="sb_av")
        if bi % 2 == 0:
            nc.scalar.copy(out=sb_av, in_=av[0:65, :, :])
        else:
            nc.vector.tensor_copy(out=sb_av, in_=av[0:65, :, :])
        den2 = nrm.tile([2, 512], f32, tag="den")
        nc.sync.dma_start(out=den2[0:1, :], in_=sb_av[64:65, 0, :])
        nc.sync.dma_start(out=den2[1:2, :], in_=sb_av[64:65, 1, :])
        r2 = nrm.tile([2, 512], bf16, tag="r")
        nc.vector._custom_dve(RECIPROCAL_APPROX_FAST, out=r2, in0=den2,
                              s0=rc["s0"], s1=rc["s1"], imm2=rc["imm2"])
        nc.sync.dma_start(out=rsc_d[bi], in_=r2)
        rbc = nrm.tile([64, 2, 512], bf16, tag="rbc")
        nc.sync.dma_start(out=rbc[:, 0, :],
                          in_=_broadcast_row(rsc_d[bi, 0:1, :], 64))
        nc.sync.dma_start(out=rbc[:, 1, :],
                          in_=_broadcast_row(rsc_d[bi, 1:2, :], 64))
        nc.gpsimd.tensor_tensor(out=attn_sb[0:64, hp, t1s],
                                in0=sb_av[0:64, 0, :], in1=rbc[:, 0, :],
                                op=ALU.mult)
        tmp = nrm.tile([64, 512], bf16, tag="tmp")
        nc.gpsimd.tensor_tensor(out=tmp, in0=sb_av[0:64, 1, :],
                                in1=rbc[:, 1, :], op=ALU.mult)
        nc.sync.dma_start(out=attn_sb[64:128, hp, t1s], in_=tmp)

    c_queue = []

    def emit_c_unit(tt2):
        """Output projection for one 128-token tile (both 512-wide halves)."""
        ps_o = ps8.tile([128, 2, 512], f32, tag="sc", name="ps_o")
        for et in range(2):
            es = slice(et * 512, (et + 1) * 512)
            for dt in range(4):
                nc.tensor.matmul(
                    ps_o[:, et, :],
                    attn_sb[:, dt, tt2 * 128:(tt2 + 1) * 128],
                    wo_sb[:, dt, es],
                    start=(dt == 0), stop=(dt == 3),
                )
        o_t = op_.tile([128, 1024], f32, tag="o")
        if tt2 % 2 == 0:
            nc.scalar.copy(out=o_t, in_=ps_o.rearrange("p a b -> p (a b)"))
        else:
            nc.vector.tensor_copy(out=o_t, in_=ps_o.rearrange("p a b -> p (a b)"))
        nc.sync.dma_start(
            out=out_d[tt2 * 128:(tt2 + 1) * 128, 0:512], in_=o_t[:, 0:512])
        nc.sync.dma_start(
            out=out_d[tt2 * 128:(tt2 + 1) * 128, 512:1024], in_=o_t[:, 512:1024])

    state = {"next": 0}

    def do_step():
        step = state["next"]
        emit_lead(step)
        if step - LAG >= 0:
            emit_trail(step - LAG)
        state["next"] = step + 1

    def try_step_b(tt):
        """Emit one early-B step (hp=0 blocks only) if its inputs exist."""
        step = state["next"]
        if step >= 4 * N_TT:
            return False
        bi, t2t = divmod(step, N_TT)
        hp, t1b = blocks[bi]
        if (hp, 0, t1b) not in qk_done or (hp, 1, t2t // 4) not in qk_done:
            return False
        if t2t > tt:  # trail needs v(t2t); stay behind the LN pipeline
            return False
        do_step()
        return True

    def finish_b():
        while state["next"] < n_steps:
            do_step()
        for step in range(n_steps, n_steps + LAG):
            emit_trail(step - LAG)
        while c_queue:
            emit_c_unit(c_queue.pop(0))

    return try_step_b, finish_b


_PROGRAM_CACHE = {}


def _get_program(mu_val: float):
    key = round(float(mu_val), 9)
    if key not in _PROGRAM_CACHE:
        _PROGRAM_CACHE[key] = _build_program(float(mu_val))
    return _PROGRAM_CACHE[key]


def make_core_inputs(sequence, ln_gamma, ln_beta, Wq, bq, Wk, bk, Wv, bv, Wo, bo, mu):
    """Host-side shard prep: per-core input dicts (gamma/beta folded into W/b)."""
    f = np.float32
    bf = _ml.bfloat16
    seq = np.asarray(sequence, f)
    g = np.asarray(ln_gamma, f)
    be = np.asarray(ln_beta, f)
    in_maps = []
    for c in range(N_CORES):
        b, grp = c // 2, c % 2
        blk = slice(W_LOC * grp, W_LOC * (grp + 1))
        Wqb = np.asarray(Wq, f)[blk]
        Wkb = np.asarray(Wk, f)[blk]
        Wvb = np.asarray(Wv, f)[blk]
        m = {
            "x": np.ascontiguousarray(seq[b]).astype(bf),
            "wqT": np.ascontiguousarray((Wqb * g[None, :]).T).astype(bf),
            "wkT": np.ascontiguousarray((Wkb * g[None, :]).T).astype(bf),
            "wvT": np.ascontiguousarray((Wvb * g[None, :]).T).astype(bf),
            "woT": np.ascontiguousarray(np.asarray(Wo, f)[:, blk].T).astype(bf),
            "bq": np.ascontiguousarray(np.asarray(bq, f)[blk] + Wqb @ be),
            "bk": np.ascontiguousarray(np.asarray(bk, f)[blk] + Wkb @ be),
            "bv": np.ascontiguousarray(np.asarray(bv, f)[blk] + Wvb @ be),
        }
        in_maps.append(m)
    return in_maps


def combine_outputs(results, bo):
    out = np.zeros((BATCH, SEQ, D_MODEL), np.float32)
    for c in range(N_CORES):
        out[c // 2] += results[c]["out"]
    out += np.asarray(bo, np.float32)[None, None, :]
    return out


def kernel(sequence, ln_gamma, ln_beta, Wq, bq, Wk, bk, Wv, bv, Wo, bo, mu,
           _trace=False):
    from concourse.bass_utils import run_bass_kernel_spmd

    mu_val = float(np.asarray(mu).reshape(-1)[0])
    nc = _get_program(mu_val)
    in_maps = make_core_inputs(sequence, ln_gamma, ln_beta, Wq, bq, Wk, bk,
                               Wv, bv, Wo, bo, mu)
    res = run_bass_kernel_spmd(nc, in_maps, list(range(N_CORES)), trace=_trace)
    out = combine_outputs(res.results, bo)
    if _trace:
        kernel.last_results = res
    return out
